# revision 42
# baseline (speedup 1.0000x reference)
"""GATv2 attention-pool kernel for 8 Trainium2 NeuronCores.

Algorithm
---------
Reference computes, per edge e with target node t(e):
    feats = q + k                                   [E, 64]
    logits[e,h] = sum_c feats[e,h*8+c] * A[c,h]     [E, 8]
    attn = segment_softmax(logits, targets)         [E, 8]
    out[n] = relu(segment_sum(q * attn))            [N, 64]

Logits are O(10) so exp() never overflows fp32/bf16; the segment-max shift
is unnecessary and softmax folds into two segment-SUMS sharing one pass:
    denom[n,h]  = sum_{e->n} exp(logits[e,h])
    pooled[n,:] = sum_{e->n} q[e,:] * exp(logits[e,h])
    out[n]      = relu(pooled[n]) / denom[n]        (relu commutes: denom>0)

Distribution: edges partitioned by target node (host-side sort), 100000
nodes split into 8 contiguous shards -> all segment reductions core-local,
no collectives.  Each shard's nodes are LPT-packed into windows of <= 32
nodes and <= 512 edges (4 subtiles of 128, ~0.6% slot padding); per
subtile the PE accumulates
    psum[32, 72] += S^T @ [q*ex | ex]
over the window's subtiles (S = host-built one-hot selector, streamed like
the data), then relu/divide once per node.

Key performance choices (vs a naive port):
- fp16 staging of q/k and bf16 ex/matmul operands halve HBM traffic and
  double DVE throughput (2x_1p mode needs 2-byte packed operands);
  rel-err vs the fp32 reference is ~8e-3 (tolerance 2e-2).
- DRAM arrays are pre-tiled on host so every DMA descriptor is one large
  contiguous run per partition.
- h-major feature columns (c outer, h inner) make the per-head exp()
  broadcast and the 1/denom broadcast middle-axis: every DVE operand
  keeps a packed innermost dim and no broadcast is ever materialized.
- the c-reduction for logits is a 3-step contiguous-halves add tree
  (tensor_reduce has no fast DVE mode; the tree runs at 2x).
- ALL elementwise work runs on DVE; ACT does exp/relu, PE the matmuls.
  Offloading to GpSimd measured slower (Q7 software ALU ~4x slower and
  any DVE<->Pool dependency creates pipeline convoys).
- 3-deep software pipeline (DMA 4 ahead, f-add 2 ahead, logits 1 ahead,
  epilogue 2 behind) so the DVE instruction queue never head-blocks.

Host work is index metadata + data layout only (argsort/packing of
targets, one-hot selector layout, gather + dtype cast of q/k rows into
slot order); all floating-point arithmetic runs on device.
"""

import os
import sys

import numpy as np

N_NODES = 100000
N_EDGES = 1600000
H = 8
C = 8
HC = H * C
N_CORES = 8
SUB = 128
WIN = 32          # nodes per window
TW = 4            # subtiles per window
CAP = TW * SUB    # max edges per window
G = 14            # windows per device group (2 psum tiles)
PWIN = 7          # windows per psum tile ((7*72)*4B < 2KB bank)

FADD_DVE_SUBTILES = 999  # f-add subtiles on DVE (rest on GpSimd).  All-DVE
                         # measured fastest: any GpSimd share re-couples the
                         # Vector pipeline to the slow Q7 engine.
DMUL_ENGINE = "vector"   # engine for out = relu(pooled) * rc


def _ensure_imports():
    try:
        import concourse.bass  # noqa: F401
    except ImportError:
        for p in ("/opt/trn_rl_repo", "/root/.axon_site/_ro/trn_rl_repo"):
            if os.path.isdir(p) and p not in sys.path:
                sys.path.insert(0, p)


def preprocess(targets):
    """Sort edges by target; LPT-pack each core's nodes into windows.

    Windows hold <= WIN nodes and <= CAP edges.  LPT (assign nodes in
    descending degree order to the least-loaded feasible window) packs to
    ~0.6% slot padding.  Returns (perms [n_cores, n_slots] edge ids (-1 =
    padding), rels [n_cores, n_slots] int8 local node id (-1 = padding),
    node_order [n_cores, n_win*WIN] node id per output row (-1 = unused),
    n_win).
    """
    import heapq

    npc = N_NODES // N_CORES
    order = np.argsort(targets, kind="stable")
    tsorted = targets[order]
    node_start = np.searchsorted(tsorted, np.arange(N_NODES + 1))
    deg = np.diff(node_start)

    def lpt(nodes, n_win):
        degs = deg[nodes]
        heap = [(0, 0, i) for i in range(n_win)]
        heapq.heapify(heap)
        assign = [[] for _ in range(n_win)]
        for nd in np.argsort(-degs, kind="stable"):
            dd = int(degs[nd])
            popped = []
            placed = False
            while heap:
                load, cnt, i = heapq.heappop(heap)
                if load + dd <= CAP and cnt + 1 <= WIN:
                    heapq.heappush(heap, (load + dd, cnt + 1, i))
                    assign[i].append(int(nodes[nd]))
                    placed = True
                    break
                popped.append((load, cnt, i))
            for p in popped:
                heapq.heappush(heap, p)
            if not placed:
                return None
        return assign

    # minimal feasible n_win per core, then re-pack all cores at the max
    packs, n_wins = [], []
    for c in range(N_CORES):
        nodes = np.arange(c * npc, (c + 1) * npc)
        n_win = int(np.ceil(max(deg[nodes].sum() / CAP, len(nodes) / WIN)))
        while True:
            a = lpt(nodes, n_win)
            if a is not None:
                break
            n_win += 1
        n_wins.append(n_win)
    n_win = max(n_wins)
    for c in range(N_CORES):
        nodes = np.arange(c * npc, (c + 1) * npc)
        a = lpt(nodes, n_win)
        assert a is not None
        packs.append(a)

    n_slots = n_win * CAP
    perms = np.full((N_CORES, n_slots), -1, dtype=np.int64)
    rels = np.full((N_CORES, n_slots), -1, dtype=np.int8)
    node_order = np.full((N_CORES, n_win * WIN), -1, dtype=np.int64)
    for c in range(N_CORES):
        for w, cur in enumerate(packs[c]):
            sb = w * CAP
            pos = 0
            for j, node in enumerate(cur):
                e0, e1 = node_start[node], node_start[node + 1]
                cnt = e1 - e0
                perms[c, sb + pos:sb + pos + cnt] = order[e0:e1]
                rels[c, sb + pos:sb + pos + cnt] = j
                pos += cnt
                node_order[c, w * WIN + j] = node
    return perms, rels, node_order, n_win


def _groups(n_win):
    gs, w0 = [], 0
    while w0 < n_win:
        g = min(G, n_win - w0)
        gs.append((w0, g))
        w0 += g
    return gs


def build_nc(n_win):
    """Build the single SPMD Bass program for one core's shard."""
    _ensure_imports()
    import concourse.bacc as bacc
    import concourse.mybir as mybir
    import concourse.tile as tile

    f32 = mybir.dt.float32
    f16 = mybir.dt.float16
    bf16 = mybir.dt.bfloat16

    groups = _groups(n_win)
    AF = mybir.ActivationFunctionType
    OP = mybir.AluOpType

    nc = bacc.Bacc("TRN2", num_devices=N_CORES)
    qkD = nc.declare_dram_parameter("qk", [SUB, n_win * TW * SUB], f16, False)
    sD = nc.declare_dram_parameter("sel", [SUB, n_win * TW * WIN], bf16,
                                   False)
    wD = nc.declare_dram_parameter("wrow", [SUB, G * TW * HC], f16, False)
    outD = nc.declare_dram_parameter(
        "out", [WIN, n_win * HC], bf16, isOutput=True)

    with tile.TileContext(nc) as tc:
        with (
            tc.tile_pool(name="const", bufs=1) as cpool,
            tc.tile_pool(name="qk", bufs=4) as qkpool,
            tc.tile_pool(name="mid", bufs=3) as midpool,
            tc.tile_pool(name="mm", bufs=3) as mmpool,
            tc.tile_pool(name="fin", bufs=3) as finpool,
            tc.tile_pool(name="psum", bufs=8, space="PSUM") as ppool,
        ):
            w_t = cpool.tile([SUB, G * TW * HC], f16)
            nc.sync.dma_start(out=w_t[:], in_=wD[:])

            def emit_load(grp):
                w0, g = grp
                Tg = TW * g
                qk_t = qkpool.tile([SUB, Tg * SUB], f16, tag=f"qk{g}")
                nc.sync.dma_start(
                    out=qk_t[:], in_=qkD[:, w0 * CAP:(w0 + g) * CAP])
                s_t = qkpool.tile([SUB, Tg, WIN], bf16, tag=f"S{g}")
                nc.sync.dma_start(
                    out=s_t[:], in_=sD[:, w0 * TW * WIN:(w0 + g) * TW * WIN])
                qk3 = qk_t[:].rearrange("p (t c) -> p t c", c=SUB)
                return {"grp": grp, "Tg": Tg, "qk3": qk3, "S": s_t}

            def emit_fadd(s):
                # split across DVE and GpSimd; deep per-tag bufs so slot
                # recycling (write-after-write) never waits on GpSimd
                Tg = s["Tg"]
                ks = min(FADD_DVE_SUBTILES, Tg)
                f_t = midpool.tile([SUB, Tg * HC], f16, tag=f"f{Tg}", bufs=3)
                fv = f_t[:].rearrange("p (t c) -> p t c", c=HC)
                if ks > 0:
                    nc.vector.tensor_add(
                        fv[:, 0:ks, :],
                        s["qk3"][:, 0:ks, 0:HC], s["qk3"][:, 0:ks, HC:2 * HC])
                if ks < Tg:
                    nc.gpsimd.tensor_add(
                        fv[:, ks:Tg, :],
                        s["qk3"][:, ks:Tg, 0:HC], s["qk3"][:, ks:Tg, HC:2 * HC])
                s["f"] = f_t

            def emit_logits(s):
                # features are h-major (c outer, h inner): the c-reduction
                # tree adds contiguous 32/16/8-element halves (2x DVE mode)
                Tg = s["Tg"]
                wf_t = midpool.tile([SUB, Tg * HC], f16, tag=f"wf{Tg}")
                nc.vector.tensor_mul(wf_t[:], s["f"][:], w_t[:, :Tg * HC])
                wfv = wf_t[:].rearrange("p (t c) -> p t c", c=HC)
                t1 = midpool.tile([SUB, Tg, 32], f16, tag=f"t1{Tg}")
                nc.vector.tensor_add(
                    t1[:], wfv[:, :, 0:32], wfv[:, :, 32:64])
                t2 = midpool.tile([SUB, Tg, 16], f16, tag=f"t2{Tg}")
                nc.vector.tensor_add(t2[:], t1[:, :, 0:16], t1[:, :, 16:32])
                lg = midpool.tile([SUB, Tg * H], f32, tag=f"lg{Tg}")
                nc.vector.tensor_add(
                    lg[:].rearrange("p (t h) -> p t h", h=H),
                    t2[:, :, 0:8], t2[:, :, 8:16])
                s["lg"] = lg

            def emit_exp(s):
                Tg = s["Tg"]
                m_t = mmpool.tile([SUB, Tg, HC + H], bf16, tag=f"M{Tg}")
                lg3 = s["lg"][:].rearrange("p (t h) -> p t h", h=H)
                nc.scalar.activation(
                    out=m_t[:, :, HC:HC + H], in_=lg3, func=AF.Exp)
                s["m"] = m_t

            def emit_qex_mm(s):
                # h-major: the per-head ex broadcast is along the MIDDLE (c)
                # axis, so every operand's innermost dim stays packed (2x)
                # and no materialized broadcast is needed.
                Tg = s["Tg"]
                m_t = s["m"]
                exv = m_t[:, :, HC:HC + H]
                nc.vector.tensor_mul(
                    m_t[:, :, 0:HC].rearrange("p t (cc h) -> p t cc h", h=H),
                    s["qk3"][:, :, 0:HC].rearrange(
                        "p t (cc h) -> p t cc h", h=H),
                    exv[:, :, None, :].to_broadcast([SUB, Tg, C, H]),
                )
                w0, g = s["grp"]
                n_ps = (g + PWIN - 1) // PWIN
                ps = []
                for pi in range(n_ps):
                    nw = min(PWIN, g - pi * PWIN)
                    p_t = ppool.tile([WIN, PWIN * (HC + H)], f32, tag="ps")
                    for wi in range(nw):
                        sub0 = (pi * PWIN + wi) * TW
                        pcols = slice(wi * (HC + H), (wi + 1) * (HC + H))
                        for t in range(TW):
                            nc.tensor.matmul(
                                p_t[:, pcols],
                                lhsT=s["S"][:, sub0 + t, :],
                                rhs=m_t[:, sub0 + t, :],
                                start=(t == 0),
                                stop=(t == TW - 1),
                            )
                    ps.append((p_t, nw))
                s["ps"] = ps

            def emit_epilogue(s):
                w0, g = s["grp"]
                po = finpool.tile([WIN, g, HC + H], bf16, tag=f"po{g}")
                off = 0
                for p_t, nw in s["ps"]:
                    nc.scalar.activation(
                        out=po[:, off:off + nw, :],
                        in_=p_t[:, :nw * (HC + H)].rearrange(
                            "p (w j) -> p w j", j=HC + H),
                        func=AF.Relu,
                    )
                    off += nw
                rc = finpool.tile([WIN, g, H], bf16, tag=f"rc{g}")
                with nc.allow_low_precision(reason="attn 1/denom in bf16"):
                    nc.vector.reciprocal(rc[:], po[:, :, HC:HC + H])
                o_t = finpool.tile([WIN, g, HC], bf16, tag=f"o{g}")
                nc.vector.tensor_mul(
                    o_t[:].rearrange("p w (cc h) -> p w cc h", h=H),
                    po[:, :, 0:HC].rearrange("p w (cc h) -> p w cc h", h=H),
                    rc[:, :, None, :].to_broadcast([WIN, g, C, H]),
                )
                nc.sync.dma_start(
                    out=outD[:, w0 * HC:(w0 + g) * HC], in_=o_t[:])

            # 3-deep software pipeline; epilogue runs 2 iterations behind so
            # no engine's first op of an iteration waits on a fresh product.
            # Steady-state per-iteration engine FIFOs:
            #   ACT:  relu(i-2), exp/expb(i)
            #   DVE:  recip(i-2), wmul/tree(i+1), fadd-share(i+2), qex(i)
            #   Pool: dmul(i-2), fadd-share(i+2)
            #   PE:   mm(i);  DMA: out(i-2), load(i+3)
            n = len(groups)
            st = [None] * n

            def stage(gi, fn):
                if 0 <= gi < n:
                    fn(st[gi])

            for gi in range(min(2, n)):
                st[gi] = emit_load(groups[gi])
            stage(0, emit_fadd)
            stage(0, emit_logits)
            for gi in range(n):
                if gi + 2 < n:
                    st[gi + 2] = emit_load(groups[gi + 2])
                stage(gi, emit_exp)
                stage(gi + 1, emit_fadd)
                stage(gi + 1, emit_logits)
                stage(gi - 2, emit_epilogue)
                stage(gi, emit_qex_mm)
                if gi - 2 >= 0:
                    st[gi - 2] = None
            stage(n - 2, emit_epilogue)
            stage(n - 1, emit_epilogue)

    nc.finalize()
    return nc


def _host_arrays(query, key, attn_kernel, targets):
    _ensure_imports()
    import concourse.mybir as mybir

    bf16 = mybir.dt.np(mybir.dt.bfloat16)
    perms, rels, node_order, n_win = preprocess(targets)
    n_slots = n_win * CAP

    # h-major feature columns on device: col c*8+h holds head-h channel-c.
    # COLPERM is an involution (8x8 transpose), so it also un-permutes.
    colperm = np.arange(HC).reshape(H, C).T.reshape(-1)
    wrow_1 = attn_kernel.reshape(-1)  # [c*8+h] = A[c,h]
    wrow = np.tile(wrow_1, (SUB, G * TW)).astype(np.float16)

    q16 = query[:, colperm].astype(np.float16)
    k16 = key[:, colperm].astype(np.float16)
    jj = np.arange(WIN, dtype=np.int8)
    in_maps = []
    for c in range(N_CORES):
        sel = perms[c]
        valid = sel >= 0
        qkc = np.zeros((n_slots, 2 * HC), dtype=np.float16)
        qkc[valid, :HC] = q16[sel[valid]]
        qkc[valid, HC:] = k16[sel[valid]]
        # tile: [slot, c] -> [p, (w t c)] with slot = (w*TW + t)*SUB + p
        qk_til = np.ascontiguousarray(
            qkc.reshape(n_win * TW, SUB, 2 * HC).transpose(1, 0, 2)
            .reshape(SUB, n_win * TW * SUB))
        # one-hot selector, pre-tiled: [p, (w t j)]
        onehot = (rels[c][:, None] == jj[None, :])  # [n_slots, WIN] bool
        s_til = np.ascontiguousarray(
            onehot.reshape(n_win * TW, SUB, WIN).transpose(1, 0, 2)
            .reshape(SUB, n_win * TW * WIN)).astype(np.float32).astype(bf16)
        in_maps.append({
            "qk": qk_til,
            "sel": s_til,
            "wrow": wrow,
        })
    return in_maps, node_order, n_win


TRACE = False          # set by test harness to capture an NTFF profile
TRACE_CORES = None
LAST_RESULTS = None    # BassKernelResults of the most recent run


def kernel(query, key, attn_kernel, targets):
    global LAST_RESULTS
    query = np.asarray(query, dtype=np.float32)
    key = np.asarray(key, dtype=np.float32)
    attn_kernel = np.asarray(attn_kernel, dtype=np.float32)
    targets = np.asarray(targets, dtype=np.int32)

    _ensure_imports()
    from concourse.bass_utils import run_bass_kernel_spmd

    in_maps, node_order, n_win = _host_arrays(
        query, key, attn_kernel, targets)
    nc = build_nc(n_win)
    res = run_bass_kernel_spmd(
        nc, in_maps, list(range(N_CORES)),
        trace=TRACE, trace_cores=TRACE_CORES,
    )
    LAST_RESULTS = res
    colperm = np.arange(HC).reshape(H, C).T.reshape(-1)
    out = np.zeros((N_NODES, HC), dtype=np.float32)
    for c in range(N_CORES):
        # out dram [WIN, n_win*HC] -> rows (w*WIN + p); cols are h-major
        oc = np.asarray(res.results[c]["out"]).astype(np.float32)
        oc = oc.reshape(WIN, n_win, HC).transpose(1, 0, 2) \
            .reshape(n_win * WIN, HC)[:, colperm]
        rows = node_order[c]
        vmask = rows >= 0
        out[rows[vmask]] = oc[vmask]

    deg = np.bincount(targets, minlength=N_NODES)
    out[deg == 0] = 0.0
    return out


# revision 44
# speedup vs baseline: 1.0175x; 1.0175x over previous
"""GATv2 attention-pool kernel for 8 Trainium2 NeuronCores.

Algorithm
---------
Reference computes, per edge e with target node t(e):
    feats = q + k                                   [E, 64]
    logits[e,h] = sum_c feats[e,h*8+c] * A[c,h]     [E, 8]
    attn = segment_softmax(logits, targets)         [E, 8]
    out[n] = relu(segment_sum(q * attn))            [N, 64]

Logits are O(10) so exp() never overflows fp32/bf16; the segment-max shift
is unnecessary and softmax folds into two segment-SUMS sharing one pass:
    denom[n,h]  = sum_{e->n} exp(logits[e,h])
    pooled[n,:] = sum_{e->n} q[e,:] * exp(logits[e,h])
    out[n]      = relu(pooled[n]) / denom[n]        (relu commutes: denom>0)

Distribution: edges partitioned by target node (host-side sort), 100000
nodes split into 8 contiguous shards -> all segment reductions core-local,
no collectives.  Each shard's nodes are LPT-packed into windows of <= 32
nodes and <= 512 edges (4 subtiles of 128, ~0.6% slot padding); per
subtile the PE accumulates
    psum[32, 72] += S^T @ [q*ex | ex]
over the window's subtiles (S = host-built one-hot selector, streamed like
the data), then relu/divide once per node.

Key performance choices (vs a naive port):
- fp16 staging of q/k and bf16 ex/matmul operands halve HBM traffic and
  double DVE throughput (2x_1p mode needs 2-byte packed operands);
  rel-err vs the fp32 reference is ~8e-3 (tolerance 2e-2).
- DRAM arrays are pre-tiled on host so every DMA descriptor is one large
  contiguous run per partition.
- h-major feature columns (c outer, h inner) make the per-head exp()
  broadcast and the 1/denom broadcast middle-axis: every DVE operand
  keeps a packed innermost dim and no broadcast is ever materialized.
- the c-reduction for logits is a 3-step contiguous-halves add tree
  (tensor_reduce has no fast DVE mode; the tree runs at 2x).
- ALL elementwise work runs on DVE; ACT does exp/relu, PE the matmuls.
  Offloading to GpSimd measured slower (Q7 software ALU ~4x slower and
  any DVE<->Pool dependency creates pipeline convoys).
- software pipeline: DMA loads run 2 group-iterations ahead, logits
  (f-add/mul/tree) 1 ahead, the epilogue 2 behind; since all elementwise
  ops share the in-order DVE queue, f-add needs no cross-engine lead and
  the queue never head-blocks.

Host work is index metadata + data layout only (argsort/packing of
targets, one-hot selector layout, gather + dtype cast of q/k rows into
slot order); all floating-point arithmetic runs on device.
"""

import os
import sys

import numpy as np

N_NODES = 100000
N_EDGES = 1600000
H = 8
C = 8
HC = H * C
N_CORES = 8
SUB = 128
WIN = 32          # nodes per window
TW = 4            # subtiles per window
CAP = TW * SUB    # max edges per window
G = 14            # windows per device group (2 psum tiles)
PWIN = 7          # windows per psum tile ((7*72)*4B < 2KB bank)

FADD_DVE_SUBTILES = 999  # f-add subtiles on DVE (rest on GpSimd).  All-DVE
                         # measured fastest: any GpSimd share re-couples the
                         # Vector pipeline to the slow Q7 engine.
DMUL_ENGINE = "vector"   # engine for out = relu(pooled) * rc


def _ensure_imports():
    try:
        import concourse.bass  # noqa: F401
    except ImportError:
        for p in ("/opt/trn_rl_repo", "/root/.axon_site/_ro/trn_rl_repo"):
            if os.path.isdir(p) and p not in sys.path:
                sys.path.insert(0, p)


def preprocess(targets):
    """Sort edges by target; LPT-pack each core's nodes into windows.

    Windows hold <= WIN nodes and <= CAP edges.  LPT (assign nodes in
    descending degree order to the least-loaded feasible window) packs to
    ~0.6% slot padding.  Returns (perms [n_cores, n_slots] edge ids (-1 =
    padding), rels [n_cores, n_slots] int8 local node id (-1 = padding),
    node_order [n_cores, n_win*WIN] node id per output row (-1 = unused),
    n_win).
    """
    import heapq

    npc = N_NODES // N_CORES
    order = np.argsort(targets, kind="stable")
    tsorted = targets[order]
    node_start = np.searchsorted(tsorted, np.arange(N_NODES + 1))
    deg = np.diff(node_start)

    def lpt(nodes, n_win):
        degs = deg[nodes]
        heap = [(0, 0, i) for i in range(n_win)]
        heapq.heapify(heap)
        assign = [[] for _ in range(n_win)]
        for nd in np.argsort(-degs, kind="stable"):
            dd = int(degs[nd])
            popped = []
            placed = False
            while heap:
                load, cnt, i = heapq.heappop(heap)
                if load + dd <= CAP and cnt + 1 <= WIN:
                    heapq.heappush(heap, (load + dd, cnt + 1, i))
                    assign[i].append(int(nodes[nd]))
                    placed = True
                    break
                popped.append((load, cnt, i))
            for p in popped:
                heapq.heappush(heap, p)
            if not placed:
                return None
        return assign

    # minimal feasible n_win per core, then re-pack all cores at the max
    packs, n_wins = [], []
    for c in range(N_CORES):
        nodes = np.arange(c * npc, (c + 1) * npc)
        n_win = int(np.ceil(max(deg[nodes].sum() / CAP, len(nodes) / WIN)))
        while True:
            a = lpt(nodes, n_win)
            if a is not None:
                break
            n_win += 1
        n_wins.append(n_win)
    n_win = max(n_wins)
    for c in range(N_CORES):
        nodes = np.arange(c * npc, (c + 1) * npc)
        a = lpt(nodes, n_win)
        assert a is not None
        packs.append(a)

    n_slots = n_win * CAP
    perms = np.full((N_CORES, n_slots), -1, dtype=np.int64)
    rels = np.full((N_CORES, n_slots), -1, dtype=np.int8)
    node_order = np.full((N_CORES, n_win * WIN), -1, dtype=np.int64)
    for c in range(N_CORES):
        for w, cur in enumerate(packs[c]):
            sb = w * CAP
            pos = 0
            for j, node in enumerate(cur):
                e0, e1 = node_start[node], node_start[node + 1]
                cnt = e1 - e0
                perms[c, sb + pos:sb + pos + cnt] = order[e0:e1]
                rels[c, sb + pos:sb + pos + cnt] = j
                pos += cnt
                node_order[c, w * WIN + j] = node
    return perms, rels, node_order, n_win


def _groups(n_win):
    gs, w0 = [], 0
    while w0 < n_win:
        g = min(G, n_win - w0)
        gs.append((w0, g))
        w0 += g
    return gs


def build_nc(n_win):
    """Build the single SPMD Bass program for one core's shard."""
    _ensure_imports()
    import concourse.bacc as bacc
    import concourse.mybir as mybir
    import concourse.tile as tile

    f32 = mybir.dt.float32
    f16 = mybir.dt.float16
    bf16 = mybir.dt.bfloat16

    groups = _groups(n_win)
    AF = mybir.ActivationFunctionType
    OP = mybir.AluOpType

    nc = bacc.Bacc("TRN2", num_devices=N_CORES)
    qkD = nc.declare_dram_parameter("qk", [SUB, n_win * TW * SUB], f16, False)
    sD = nc.declare_dram_parameter("sel", [SUB, n_win * TW * WIN], bf16,
                                   False)
    wD = nc.declare_dram_parameter("wrow", [SUB, G * TW * HC], f16, False)
    outD = nc.declare_dram_parameter(
        "out", [WIN, n_win * HC], bf16, isOutput=True)

    with tile.TileContext(nc) as tc:
        with (
            tc.tile_pool(name="const", bufs=1) as cpool,
            tc.tile_pool(name="qk", bufs=4) as qkpool,
            tc.tile_pool(name="mid", bufs=3) as midpool,
            tc.tile_pool(name="mm", bufs=3) as mmpool,
            tc.tile_pool(name="fin", bufs=3) as finpool,
            tc.tile_pool(name="psum", bufs=8, space="PSUM") as ppool,
        ):
            w_t = cpool.tile([SUB, G * TW * HC], f16)
            nc.sync.dma_start(out=w_t[:], in_=wD[:])

            def emit_load(grp):
                w0, g = grp
                Tg = TW * g
                qk_t = qkpool.tile([SUB, Tg * SUB], f16, tag=f"qk{g}")
                nc.sync.dma_start(
                    out=qk_t[:], in_=qkD[:, w0 * CAP:(w0 + g) * CAP])
                s_t = qkpool.tile([SUB, Tg, WIN], bf16, tag=f"S{g}")
                nc.sync.dma_start(
                    out=s_t[:], in_=sD[:, w0 * TW * WIN:(w0 + g) * TW * WIN])
                qk3 = qk_t[:].rearrange("p (t c) -> p t c", c=SUB)
                return {"grp": grp, "Tg": Tg, "qk3": qk3, "S": s_t}

            def emit_fadd(s):
                # split across DVE and GpSimd; deep per-tag bufs so slot
                # recycling (write-after-write) never waits on GpSimd
                Tg = s["Tg"]
                ks = min(FADD_DVE_SUBTILES, Tg)
                f_t = midpool.tile([SUB, Tg * HC], f16, tag=f"f{Tg}", bufs=3)
                fv = f_t[:].rearrange("p (t c) -> p t c", c=HC)
                if ks > 0:
                    nc.vector.tensor_add(
                        fv[:, 0:ks, :],
                        s["qk3"][:, 0:ks, 0:HC], s["qk3"][:, 0:ks, HC:2 * HC])
                if ks < Tg:
                    nc.gpsimd.tensor_add(
                        fv[:, ks:Tg, :],
                        s["qk3"][:, ks:Tg, 0:HC], s["qk3"][:, ks:Tg, HC:2 * HC])
                s["f"] = f_t

            def emit_logits(s):
                # features are h-major (c outer, h inner): the c-reduction
                # tree adds contiguous 32/16/8-element halves (2x DVE mode)
                Tg = s["Tg"]
                wf_t = midpool.tile([SUB, Tg * HC], f16, tag=f"wf{Tg}")
                nc.vector.tensor_mul(wf_t[:], s["f"][:], w_t[:, :Tg * HC])
                wfv = wf_t[:].rearrange("p (t c) -> p t c", c=HC)
                t1 = midpool.tile([SUB, Tg, 32], f16, tag=f"t1{Tg}")
                nc.vector.tensor_add(
                    t1[:], wfv[:, :, 0:32], wfv[:, :, 32:64])
                t2 = midpool.tile([SUB, Tg, 16], f16, tag=f"t2{Tg}")
                nc.vector.tensor_add(t2[:], t1[:, :, 0:16], t1[:, :, 16:32])
                lg = midpool.tile([SUB, Tg * H], f16, tag=f"lg{Tg}")
                nc.vector.tensor_add(
                    lg[:].rearrange("p (t h) -> p t h", h=H),
                    t2[:, :, 0:8], t2[:, :, 8:16])
                s["lg"] = lg

            def emit_exp(s):
                Tg = s["Tg"]
                m_t = mmpool.tile([SUB, Tg, HC + H], bf16, tag=f"M{Tg}")
                lg3 = s["lg"][:].rearrange("p (t h) -> p t h", h=H)
                nc.scalar.activation(
                    out=m_t[:, :, HC:HC + H], in_=lg3, func=AF.Exp)
                s["m"] = m_t

            def emit_qex_mm(s):
                # h-major: the per-head ex broadcast is along the MIDDLE (c)
                # axis, so every operand's innermost dim stays packed (2x)
                # and no materialized broadcast is needed.
                Tg = s["Tg"]
                m_t = s["m"]
                exv = m_t[:, :, HC:HC + H]
                nc.vector.tensor_mul(
                    m_t[:, :, 0:HC].rearrange("p t (cc h) -> p t cc h", h=H),
                    s["qk3"][:, :, 0:HC].rearrange(
                        "p t (cc h) -> p t cc h", h=H),
                    exv[:, :, None, :].to_broadcast([SUB, Tg, C, H]),
                )
                w0, g = s["grp"]
                n_ps = (g + PWIN - 1) // PWIN
                ps = []
                for pi in range(n_ps):
                    nw = min(PWIN, g - pi * PWIN)
                    p_t = ppool.tile([WIN, PWIN * (HC + H)], f32, tag="ps")
                    for wi in range(nw):
                        sub0 = (pi * PWIN + wi) * TW
                        pcols = slice(wi * (HC + H), (wi + 1) * (HC + H))
                        for t in range(TW):
                            nc.tensor.matmul(
                                p_t[:, pcols],
                                lhsT=s["S"][:, sub0 + t, :],
                                rhs=m_t[:, sub0 + t, :],
                                start=(t == 0),
                                stop=(t == TW - 1),
                            )
                    ps.append((p_t, nw))
                s["ps"] = ps

            def emit_epilogue(s):
                w0, g = s["grp"]
                po = finpool.tile([WIN, g, HC + H], bf16, tag=f"po{g}")
                off = 0
                for p_t, nw in s["ps"]:
                    nc.scalar.activation(
                        out=po[:, off:off + nw, :],
                        in_=p_t[:, :nw * (HC + H)].rearrange(
                            "p (w j) -> p w j", j=HC + H),
                        func=AF.Relu,
                    )
                    off += nw
                rc = finpool.tile([WIN, g, H], bf16, tag=f"rc{g}")
                with nc.allow_low_precision(reason="attn 1/denom in bf16"):
                    nc.vector.reciprocal(rc[:], po[:, :, HC:HC + H])
                o_t = finpool.tile([WIN, g, HC], bf16, tag=f"o{g}")
                nc.vector.tensor_mul(
                    o_t[:].rearrange("p w (cc h) -> p w cc h", h=H),
                    po[:, :, 0:HC].rearrange("p w (cc h) -> p w cc h", h=H),
                    rc[:, :, None, :].to_broadcast([WIN, g, C, H]),
                )
                nc.sync.dma_start(
                    out=outD[:, w0 * HC:(w0 + g) * HC], in_=o_t[:])

            # 3-deep software pipeline; epilogue runs 2 iterations behind so
            # no engine's first op of an iteration waits on a fresh product.
            # Steady-state per-iteration engine FIFOs:
            #   ACT:  relu(i-2), exp/expb(i)
            #   DVE:  recip(i-2), wmul/tree(i+1), fadd-share(i+2), qex(i)
            #   Pool: dmul(i-2), fadd-share(i+2)
            #   PE:   mm(i);  DMA: out(i-2), load(i+3)
            n = len(groups)
            st = [None] * n

            def stage(gi, fn):
                if 0 <= gi < n:
                    fn(st[gi])

            for gi in range(min(2, n)):
                st[gi] = emit_load(groups[gi])
            stage(0, emit_fadd)
            stage(0, emit_logits)
            for gi in range(n):
                if gi + 2 < n:
                    st[gi + 2] = emit_load(groups[gi + 2])
                stage(gi, emit_exp)
                stage(gi + 1, emit_fadd)
                stage(gi + 1, emit_logits)
                stage(gi - 2, emit_epilogue)
                stage(gi, emit_qex_mm)
                if gi - 2 >= 0:
                    st[gi - 2] = None
            stage(n - 2, emit_epilogue)
            stage(n - 1, emit_epilogue)

    nc.finalize()
    return nc


def _host_arrays(query, key, attn_kernel, targets):
    _ensure_imports()
    import concourse.mybir as mybir

    bf16 = mybir.dt.np(mybir.dt.bfloat16)
    perms, rels, node_order, n_win = preprocess(targets)
    n_slots = n_win * CAP

    # h-major feature columns on device: col c*8+h holds head-h channel-c.
    # COLPERM is an involution (8x8 transpose), so it also un-permutes.
    colperm = np.arange(HC).reshape(H, C).T.reshape(-1)
    wrow_1 = attn_kernel.reshape(-1)  # [c*8+h] = A[c,h]
    wrow = np.tile(wrow_1, (SUB, G * TW)).astype(np.float16)

    q16 = query[:, colperm].astype(np.float16)
    k16 = key[:, colperm].astype(np.float16)
    jj = np.arange(WIN, dtype=np.int8)
    in_maps = []
    for c in range(N_CORES):
        sel = perms[c]
        valid = sel >= 0
        qkc = np.zeros((n_slots, 2 * HC), dtype=np.float16)
        qkc[valid, :HC] = q16[sel[valid]]
        qkc[valid, HC:] = k16[sel[valid]]
        # tile: [slot, c] -> [p, (w t c)] with slot = (w*TW + t)*SUB + p
        qk_til = np.ascontiguousarray(
            qkc.reshape(n_win * TW, SUB, 2 * HC).transpose(1, 0, 2)
            .reshape(SUB, n_win * TW * SUB))
        # one-hot selector, pre-tiled: [p, (w t j)]
        onehot = (rels[c][:, None] == jj[None, :])  # [n_slots, WIN] bool
        s_til = np.ascontiguousarray(
            onehot.reshape(n_win * TW, SUB, WIN).transpose(1, 0, 2)
            .reshape(SUB, n_win * TW * WIN)).astype(np.float32).astype(bf16)
        in_maps.append({
            "qk": qk_til,
            "sel": s_til,
            "wrow": wrow,
        })
    return in_maps, node_order, n_win


TRACE = False          # set by test harness to capture an NTFF profile
TRACE_CORES = None
LAST_RESULTS = None    # BassKernelResults of the most recent run


def kernel(query, key, attn_kernel, targets):
    global LAST_RESULTS
    query = np.asarray(query, dtype=np.float32)
    key = np.asarray(key, dtype=np.float32)
    attn_kernel = np.asarray(attn_kernel, dtype=np.float32)
    targets = np.asarray(targets, dtype=np.int32)

    _ensure_imports()
    from concourse.bass_utils import run_bass_kernel_spmd

    in_maps, node_order, n_win = _host_arrays(
        query, key, attn_kernel, targets)
    nc = build_nc(n_win)
    res = run_bass_kernel_spmd(
        nc, in_maps, list(range(N_CORES)),
        trace=TRACE, trace_cores=TRACE_CORES,
    )
    LAST_RESULTS = res
    colperm = np.arange(HC).reshape(H, C).T.reshape(-1)
    out = np.zeros((N_NODES, HC), dtype=np.float32)
    for c in range(N_CORES):
        # out dram [WIN, n_win*HC] -> rows (w*WIN + p); cols are h-major
        oc = np.asarray(res.results[c]["out"]).astype(np.float32)
        oc = oc.reshape(WIN, n_win, HC).transpose(1, 0, 2) \
            .reshape(n_win * WIN, HC)[:, colperm]
        rows = node_order[c]
        vmask = rows >= 0
        out[rows[vmask]] = oc[vmask]

    deg = np.bincount(targets, minlength=N_NODES)
    out[deg == 0] = 0.0
    return out


# revision 47
# speedup vs baseline: 1.0746x; 1.0562x over previous
"""GATv2 attention-pool kernel for 8 Trainium2 NeuronCores.

Algorithm
---------
Reference computes, per edge e with target node t(e):
    feats = q + k                                   [E, 64]
    logits[e,h] = sum_c feats[e,h*8+c] * A[c,h]     [E, 8]
    attn = segment_softmax(logits, targets)         [E, 8]
    out[n] = relu(segment_sum(q * attn))            [N, 64]

Logits are O(10) so exp() never overflows fp32/bf16; the segment-max shift
is unnecessary and softmax folds into two segment-SUMS sharing one pass:
    denom[n,h]  = sum_{e->n} exp(logits[e,h])
    pooled[n,:] = sum_{e->n} q[e,:] * exp(logits[e,h])
    out[n]      = relu(pooled[n]) / denom[n]        (relu commutes: denom>0)

Distribution: edges partitioned by target node (host-side sort), 100000
nodes split into 8 contiguous shards -> all segment reductions core-local,
no collectives.  Each shard's nodes are LPT-packed into windows of <= 32
nodes and <= 512 edges (4 subtiles of 128, ~0.6% slot padding); per
subtile the PE accumulates
    psum[32, 72] += S^T @ [q*ex | ex]
over the window's subtiles (S = host-built one-hot selector, streamed like
the data), then relu/divide once per node.

Key performance choices (vs a naive port):
- fp16 staging of q/k and bf16 ex/matmul operands halve HBM traffic and
  double DVE throughput (2x_1p mode needs 2-byte packed operands);
  rel-err vs the fp32 reference is ~8e-3 (tolerance 2e-2).
- DRAM arrays are pre-tiled on host so every DMA descriptor is one large
  contiguous run per partition.
- h-major feature columns (c outer, h inner) make the per-head exp()
  broadcast and the 1/denom broadcast middle-axis: every DVE operand
  keeps a packed innermost dim and no broadcast is ever materialized.
- the c-reduction for logits is a 3-step contiguous-halves add tree
  (tensor_reduce has no fast DVE mode; the tree runs at 2x).
- ALL elementwise work runs on DVE; ACT does exp/relu, PE the matmuls.
  Offloading to GpSimd measured slower (Q7 software ALU ~4x slower and
  any DVE<->Pool dependency creates pipeline convoys).
- software pipeline: DMA loads run 2 group-iterations ahead, logits
  (f-add/mul/tree) 1 ahead, the epilogue 2 behind; since all elementwise
  ops share the in-order DVE queue, f-add needs no cross-engine lead and
  the queue never head-blocks.

Host work is index metadata + data layout only (argsort/packing of
targets, one-hot selector layout, gather + dtype cast of q/k rows into
slot order); all floating-point arithmetic runs on device.
"""

import os
import sys

import numpy as np

N_NODES = 100000
N_EDGES = 1600000
H = 8
C = 8
HC = H * C
N_CORES = 8
SUB = 128
WIN = 32          # nodes per window
TW = 4            # subtiles per window
CAP = TW * SUB    # max edges per window
G = 14            # windows per device group (2 psum tiles)
PWIN = 7          # windows per psum tile ((7*72)*4B < 2KB bank)

FADD_DVE_SUBTILES = 999  # f-add subtiles on DVE (rest on GpSimd).  All-DVE
                         # measured fastest: any GpSimd share re-couples the
                         # Vector pipeline to the slow Q7 engine.
DMUL_ENGINE = "vector"   # engine for out = relu(pooled) * rc


def _ensure_imports():
    try:
        import concourse.bass  # noqa: F401
    except ImportError:
        for p in ("/opt/trn_rl_repo", "/root/.axon_site/_ro/trn_rl_repo"):
            if os.path.isdir(p) and p not in sys.path:
                sys.path.insert(0, p)


def preprocess(targets):
    """Sort edges by target; LPT-pack each core's nodes into windows.

    Windows hold <= WIN nodes and <= CAP edges.  LPT (assign nodes in
    descending degree order to the least-loaded feasible window) packs to
    ~0.6% slot padding.  Returns (perms [n_cores, n_slots] edge ids (-1 =
    padding), rels [n_cores, n_slots] int8 local node id (-1 = padding),
    node_order [n_cores, n_win*WIN] node id per output row (-1 = unused),
    n_win).
    """
    import heapq

    npc = N_NODES // N_CORES
    order = np.argsort(targets, kind="stable")
    tsorted = targets[order]
    node_start = np.searchsorted(tsorted, np.arange(N_NODES + 1))
    deg = np.diff(node_start)

    def lpt(nodes, n_win):
        degs = deg[nodes]
        heap = [(0, 0, i) for i in range(n_win)]
        heapq.heapify(heap)
        assign = [[] for _ in range(n_win)]
        for nd in np.argsort(-degs, kind="stable"):
            dd = int(degs[nd])
            popped = []
            placed = False
            while heap:
                load, cnt, i = heapq.heappop(heap)
                if load + dd <= CAP and cnt + 1 <= WIN:
                    heapq.heappush(heap, (load + dd, cnt + 1, i))
                    assign[i].append(int(nodes[nd]))
                    placed = True
                    break
                popped.append((load, cnt, i))
            for p in popped:
                heapq.heappush(heap, p)
            if not placed:
                return None
        return assign

    # minimal feasible n_win per core, then re-pack all cores at the max
    packs, n_wins = [], []
    for c in range(N_CORES):
        nodes = np.arange(c * npc, (c + 1) * npc)
        n_win = int(np.ceil(max(deg[nodes].sum() / CAP, len(nodes) / WIN)))
        while True:
            a = lpt(nodes, n_win)
            if a is not None:
                break
            n_win += 1
        n_wins.append(n_win)
    n_win = max(n_wins)
    for c in range(N_CORES):
        nodes = np.arange(c * npc, (c + 1) * npc)
        a = lpt(nodes, n_win)
        assert a is not None
        packs.append(a)

    n_slots = n_win * CAP
    perms = np.full((N_CORES, n_slots), -1, dtype=np.int64)
    rels = np.full((N_CORES, n_slots), -1, dtype=np.int8)
    node_order = np.full((N_CORES, n_win * WIN), -1, dtype=np.int64)
    for c in range(N_CORES):
        for w, cur in enumerate(packs[c]):
            sb = w * CAP
            pos = 0
            for j, node in enumerate(cur):
                e0, e1 = node_start[node], node_start[node + 1]
                cnt = e1 - e0
                perms[c, sb + pos:sb + pos + cnt] = order[e0:e1]
                rels[c, sb + pos:sb + pos + cnt] = j
                pos += cnt
                node_order[c, w * WIN + j] = node
    return perms, rels, node_order, n_win


def _groups(n_win):
    gs, w0 = [], 0
    while w0 < n_win:
        g = min(G, n_win - w0)
        gs.append((w0, g))
        w0 += g
    return gs


def build_nc(n_win):
    """Build the single SPMD Bass program for one core's shard."""
    _ensure_imports()
    import concourse.bacc as bacc
    import concourse.mybir as mybir
    import concourse.tile as tile

    f32 = mybir.dt.float32
    f16 = mybir.dt.float16
    bf16 = mybir.dt.bfloat16

    groups = _groups(n_win)
    AF = mybir.ActivationFunctionType
    OP = mybir.AluOpType

    nc = bacc.Bacc("TRN2", num_devices=N_CORES)
    qkD = nc.declare_dram_parameter("qk", [SUB, n_win * TW * SUB], f16, False)
    sD = nc.declare_dram_parameter("sel", [SUB, n_win * TW * WIN], bf16,
                                   False)
    wD = nc.declare_dram_parameter("wrow", [SUB, G * TW * HC], f16, False)
    outD = nc.declare_dram_parameter(
        "out", [WIN, n_win * HC], bf16, isOutput=True)

    with tile.TileContext(nc) as tc:
        with (
            tc.tile_pool(name="const", bufs=1) as cpool,
            tc.tile_pool(name="qk", bufs=4) as qkpool,
            tc.tile_pool(name="mid", bufs=3) as midpool,
            tc.tile_pool(name="mm", bufs=3) as mmpool,
            tc.tile_pool(name="fin", bufs=3) as finpool,
            tc.tile_pool(name="psum", bufs=8, space="PSUM") as ppool,
        ):
            w_t = cpool.tile([SUB, G * TW * HC], f16)
            nc.sync.dma_start(out=w_t[:], in_=wD[:])

            def emit_load(grp):
                w0, g = grp
                Tg = TW * g
                qk_t = qkpool.tile([SUB, Tg * SUB], f16, tag=f"qk{g}")
                nc.sync.dma_start(
                    out=qk_t[:], in_=qkD[:, w0 * CAP:(w0 + g) * CAP])
                s_t = qkpool.tile([SUB, Tg, WIN], bf16, tag=f"S{g}")
                nc.sync.dma_start(
                    out=s_t[:], in_=sD[:, w0 * TW * WIN:(w0 + g) * TW * WIN])
                qk3 = qk_t[:].rearrange("p (t c) -> p t c", c=SUB)
                return {"grp": grp, "Tg": Tg, "qk3": qk3, "S": s_t}

            def emit_fadd(s):
                # split across DVE and GpSimd; deep per-tag bufs so slot
                # recycling (write-after-write) never waits on GpSimd
                Tg = s["Tg"]
                ks = min(FADD_DVE_SUBTILES, Tg)
                f_t = midpool.tile([SUB, Tg * HC], f16, tag=f"f{Tg}", bufs=3)
                fv = f_t[:].rearrange("p (t c) -> p t c", c=HC)
                if ks > 0:
                    nc.vector.tensor_add(
                        fv[:, 0:ks, :],
                        s["qk3"][:, 0:ks, 0:HC], s["qk3"][:, 0:ks, HC:2 * HC])
                if ks < Tg:
                    nc.gpsimd.tensor_add(
                        fv[:, ks:Tg, :],
                        s["qk3"][:, ks:Tg, 0:HC], s["qk3"][:, ks:Tg, HC:2 * HC])
                s["f"] = f_t

            def emit_logits(s):
                # features are h-major (c outer, h inner): the c-reduction
                # tree adds contiguous 32/16/8-element halves (2x DVE mode)
                Tg = s["Tg"]
                wf_t = midpool.tile([SUB, Tg * HC], f16, tag=f"wf{Tg}")
                nc.vector.tensor_mul(wf_t[:], s["f"][:], w_t[:, :Tg * HC])
                wfv = wf_t[:].rearrange("p (t c) -> p t c", c=HC)
                t1 = midpool.tile([SUB, Tg, 32], f16, tag=f"t1{Tg}")
                nc.vector.tensor_add(
                    t1[:], wfv[:, :, 0:32], wfv[:, :, 32:64])
                t2 = midpool.tile([SUB, Tg, 16], f16, tag=f"t2{Tg}")
                nc.vector.tensor_add(t2[:], t1[:, :, 0:16], t1[:, :, 16:32])
                lg = midpool.tile([SUB, Tg * H], f16, tag=f"lg{Tg}")
                nc.vector.tensor_add(
                    lg[:].rearrange("p (t h) -> p t h", h=H),
                    t2[:, :, 0:8], t2[:, :, 8:16])
                s["lg"] = lg

            def emit_exp(s):
                Tg = s["Tg"]
                m_t = mmpool.tile([SUB, Tg, HC + H], bf16, tag=f"M{Tg}")
                lg3 = s["lg"][:].rearrange("p (t h) -> p t h", h=H)
                nc.scalar.activation(
                    out=m_t[:, :, HC:HC + H], in_=lg3, func=AF.Exp)
                s["m"] = m_t

            def emit_qex_mm(s):
                # h-major: the per-head ex broadcast is along the MIDDLE (c)
                # axis, so every operand's innermost dim stays packed (2x)
                # and no materialized broadcast is needed.
                Tg = s["Tg"]
                m_t = s["m"]
                exv = m_t[:, :, HC:HC + H]
                nc.vector.tensor_mul(
                    m_t[:, :, 0:HC].rearrange("p t (cc h) -> p t cc h", h=H),
                    s["qk3"][:, :, 0:HC].rearrange(
                        "p t (cc h) -> p t cc h", h=H),
                    exv[:, :, None, :].to_broadcast([SUB, Tg, C, H]),
                )
                w0, g = s["grp"]
                n_ps = (g + PWIN - 1) // PWIN
                ps = []
                for pi in range(n_ps):
                    nw = min(PWIN, g - pi * PWIN)
                    p_t = ppool.tile([WIN, PWIN * (HC + H)], f32, tag="ps")
                    for wi in range(nw):
                        sub0 = (pi * PWIN + wi) * TW
                        pcols = slice(wi * (HC + H), (wi + 1) * (HC + H))
                        for t in range(TW):
                            nc.tensor.matmul(
                                p_t[:, pcols],
                                lhsT=s["S"][:, sub0 + t, :],
                                rhs=m_t[:, sub0 + t, :],
                                start=(t == 0),
                                stop=(t == TW - 1),
                            )
                    ps.append((p_t, nw))
                s["ps"] = ps

            def emit_epilogue(s):
                w0, g = s["grp"]
                po = finpool.tile([WIN, g, HC + H], bf16, tag=f"po{g}")
                off = 0
                for p_t, nw in s["ps"]:
                    nc.scalar.activation(
                        out=po[:, off:off + nw, :],
                        in_=p_t[:, :nw * (HC + H)].rearrange(
                            "p (w j) -> p w j", j=HC + H),
                        func=AF.Relu,
                    )
                    off += nw
                # 1/denom via the single-pass approx (~18 bits, plenty for
                # the bf16 result); it needs fp32 in/out, staged through the
                # idle ACT engine.  denom>0 for every real node (no
                # zero-degree nodes reach here; padded rows are host-masked).
                dn = finpool.tile([WIN, g, H], f32, tag=f"dn{g}")
                nc.scalar.activation(out=dn[:], in_=po[:, :, HC:HC + H],
                                     func=AF.Copy)
                rcf = finpool.tile([WIN, g, H], f32, tag=f"rcf{g}")
                nc.vector.reciprocal_approx_fast(out=rcf[:], in_=dn[:])
                rc = finpool.tile([WIN, g, H], bf16, tag=f"rc{g}")
                nc.scalar.activation(out=rc[:], in_=rcf[:], func=AF.Copy)
                s["po"], s["rc"] = po, rc

            def emit_epi_b(s):
                w0, g = s["grp"]
                o_t = finpool.tile([WIN, g, HC], bf16, tag=f"o{g}")
                nc.vector.tensor_mul(
                    o_t[:].rearrange("p w (cc h) -> p w cc h", h=H),
                    s["po"][:, :, 0:HC].rearrange(
                        "p w (cc h) -> p w cc h", h=H),
                    s["rc"][:, :, None, :].to_broadcast([WIN, g, C, H]),
                )
                nc.sync.dma_start(
                    out=outD[:, w0 * HC:(w0 + g) * HC], in_=o_t[:])

            # 3-deep software pipeline; epilogue runs 2 iterations behind so
            # no engine's first op of an iteration waits on a fresh product.
            # Steady-state per-iteration engine FIFOs:
            #   ACT:  relu(i-2), exp/expb(i)
            #   DVE:  recip(i-2), wmul/tree(i+1), fadd-share(i+2), qex(i)
            #   Pool: dmul(i-2), fadd-share(i+2)
            #   PE:   mm(i);  DMA: out(i-2), load(i+3)
            n = len(groups)
            st = [None] * n

            def stage(gi, fn):
                if 0 <= gi < n:
                    fn(st[gi])

            for gi in range(min(2, n)):
                st[gi] = emit_load(groups[gi])
            stage(0, emit_fadd)
            stage(0, emit_logits)
            for gi in range(n):
                if gi + 2 < n:
                    st[gi + 2] = emit_load(groups[gi + 2])
                stage(gi, emit_exp)
                stage(gi + 1, emit_fadd)
                stage(gi + 1, emit_logits)
                stage(gi - 2, emit_epilogue)
                stage(gi, emit_qex_mm)
                stage(gi - 2, emit_epi_b)
                if gi - 2 >= 0:
                    st[gi - 2] = None
            for gi in (n - 2, n - 1):
                stage(gi, emit_epilogue)
                stage(gi, emit_epi_b)

    nc.finalize()
    return nc


def _host_arrays(query, key, attn_kernel, targets):
    _ensure_imports()
    import concourse.mybir as mybir

    bf16 = mybir.dt.np(mybir.dt.bfloat16)
    perms, rels, node_order, n_win = preprocess(targets)
    n_slots = n_win * CAP

    # h-major feature columns on device: col c*8+h holds head-h channel-c.
    # COLPERM is an involution (8x8 transpose), so it also un-permutes.
    colperm = np.arange(HC).reshape(H, C).T.reshape(-1)
    wrow_1 = attn_kernel.reshape(-1)  # [c*8+h] = A[c,h]
    wrow = np.tile(wrow_1, (SUB, G * TW)).astype(np.float16)

    q16 = query[:, colperm].astype(np.float16)
    k16 = key[:, colperm].astype(np.float16)
    jj = np.arange(WIN, dtype=np.int8)
    in_maps = []
    for c in range(N_CORES):
        sel = perms[c]
        valid = sel >= 0
        qkc = np.zeros((n_slots, 2 * HC), dtype=np.float16)
        qkc[valid, :HC] = q16[sel[valid]]
        qkc[valid, HC:] = k16[sel[valid]]
        # tile: [slot, c] -> [p, (w t c)] with slot = (w*TW + t)*SUB + p
        qk_til = np.ascontiguousarray(
            qkc.reshape(n_win * TW, SUB, 2 * HC).transpose(1, 0, 2)
            .reshape(SUB, n_win * TW * SUB))
        # one-hot selector, pre-tiled: [p, (w t j)]
        onehot = (rels[c][:, None] == jj[None, :])  # [n_slots, WIN] bool
        s_til = np.ascontiguousarray(
            onehot.reshape(n_win * TW, SUB, WIN).transpose(1, 0, 2)
            .reshape(SUB, n_win * TW * WIN)).astype(np.float32).astype(bf16)
        in_maps.append({
            "qk": qk_til,
            "sel": s_til,
            "wrow": wrow,
        })
    return in_maps, node_order, n_win


TRACE = False          # set by test harness to capture an NTFF profile
TRACE_CORES = None
LAST_RESULTS = None    # BassKernelResults of the most recent run


def kernel(query, key, attn_kernel, targets):
    global LAST_RESULTS
    query = np.asarray(query, dtype=np.float32)
    key = np.asarray(key, dtype=np.float32)
    attn_kernel = np.asarray(attn_kernel, dtype=np.float32)
    targets = np.asarray(targets, dtype=np.int32)

    _ensure_imports()
    from concourse.bass_utils import run_bass_kernel_spmd

    in_maps, node_order, n_win = _host_arrays(
        query, key, attn_kernel, targets)
    nc = build_nc(n_win)
    res = run_bass_kernel_spmd(
        nc, in_maps, list(range(N_CORES)),
        trace=TRACE, trace_cores=TRACE_CORES,
    )
    LAST_RESULTS = res
    colperm = np.arange(HC).reshape(H, C).T.reshape(-1)
    out = np.zeros((N_NODES, HC), dtype=np.float32)
    for c in range(N_CORES):
        # out dram [WIN, n_win*HC] -> rows (w*WIN + p); cols are h-major
        oc = np.asarray(res.results[c]["out"]).astype(np.float32)
        oc = oc.reshape(WIN, n_win, HC).transpose(1, 0, 2) \
            .reshape(n_win * WIN, HC)[:, colperm]
        rows = node_order[c]
        vmask = rows >= 0
        out[rows[vmask]] = oc[vmask]

    deg = np.bincount(targets, minlength=N_NODES)
    out[deg == 0] = 0.0
    return out
